# revision 11
# baseline (speedup 1.0000x reference)
"""Maxish pooling kernel for Trainium2 (8 NeuronCores, data-parallel).

Reference math (per row of length N):
    m  = max(x)
    rt = (x - m) / (m + 1e-8)
    pos = m * sum(exp(2*rt)) / sum(exp(rt))     # for scale s == 1
    out = m > 0 ? pos : (m < 0 ? m : 0)

Identity: rt = x*(1/m) - 1 exactly, so u = exp(rt) = Exp(y - 1) with
y = x*r.  Both sums come from one bn_stats pass per row-pair over u
read pair-interleaved (even/odd streams -> per-row mean/M2):
sum u = N*mu, sum u^2 = M2 + N*mu^2.

Work split (per chunk of [128, G=16 rows, 256], all rates measured):
  gpsimd: pairwise-max tree levels 1+2 (tensor_tensor max), plus the
          broadcast normalize y = x*r for `k_g` row-pairs
  DVE:    max tree level 3 (tensor_reduce), reciprocal, normalize for
          `k_q` pairs, bn_stats for all pairs
  ACT:    fused exp (scale=r, bias=-1, per tile) for `k_f` pairs; one
          big contiguous exp over the remaining span
Layout: chunk = 2048 consecutive rows; partition p holds rows
c*2048 + p*16 + r (16KB contiguous per partition line).
"""

import numpy as np

P = 128
N = 256


def _build(n_rows: int, s: float, G: int = 16, x_bufs: int = 3,
           y_bufs: int = 2, u_bufs: int = 2, sc_bufs: int = 2,
           k_f: int = 6, k_g: int = 1,
           dt_u_str: str = "bf16", dt_bst_str: str = "f32",
           max_tree: int = 2):
    from concourse import bacc, mybir
    from concourse.tile import TileContext

    f32 = mybir.dt.float32
    dts = {"f32": mybir.dt.float32, "bf16": mybir.dt.bfloat16}
    dt_u = dts[dt_u_str]
    dt_bst = dts[dt_bst_str]
    Act = mybir.ActivationFunctionType
    Alu = mybir.AluOpType
    Ax = mybir.AxisListType

    assert n_rows % (P * G) == 0
    T = n_rows // P          # rows per partition
    C = T // G               # chunks
    H = G // 2               # row-pairs per chunk
    fast = (s == 1.0)
    if not fast:
        k_f = 0              # generic path: no fused tiles
    k_f = min(k_f, H)
    k_g = min(k_g, H - k_f)  # gpsimd-normalized pairs
    # remaining pairs use DVE normalize
    nb = H - k_f             # pairs needing y + big exp

    nc = bacc.Bacc("TRN2", target_bir_lowering=False, debug=False,
                   num_devices=8)
    x_d = nc.declare_dram_parameter("x", [n_rows, N], f32, isOutput=False)
    out_d = nc.declare_dram_parameter("out", [n_rows], f32, isOutput=True)

    with TileContext(nc) as tc:
        with (
            tc.tile_pool(name="xp", bufs=x_bufs) as xp,
            tc.tile_pool(name="yp", bufs=y_bufs) as yp,
            tc.tile_pool(name="up", bufs=u_bufs) as up,
            tc.tile_pool(name="scp", bufs=sc_bufs) as scp,
            tc.tile_pool(name="stat", bufs=1) as statp,
        ):
            M = statp.tile([P, T], f32, tag="M")
            Rv = statp.tile([P, T], f32, tag="Rv")
            BST = statp.tile([P, (T // 2) * 6], dt_bst, tag="BST")
            BIAS = statp.tile([P, 2], f32, tag="BIAS")
            nc.vector.memset(BIAS[:, 0:1], -float(s))
            nc.vector.memset(BIAS[:, 1:2], -(1.0 + float(s)))
            if not fast:
                S1 = statp.tile([P, T], f32, tag="S1")
                S2 = statp.tile([P, T], f32, tag="S2")

            for c in range(C):
                cols = slice(c * G, (c + 1) * G)
                xt = xp.tile([P, G * N], f32, tag="x")
                src = x_d[c * G * P:(c + 1) * G * P, :].rearrange(
                    "(p r) n -> p r n", p=P)
                nc.sync.dma_start(
                    out=xt[:].rearrange("p (r n) -> p r n", n=N), in_=src)
                x3 = xt[:].rearrange("p (r n) -> p r n", n=N)

                mg = M[:, cols]
                if max_tree == 2:
                    l1 = scp.tile([P, G * (N // 2)], f32, tag="l1")
                    l13 = l1[:].rearrange("p (r n) -> p r n", n=N // 2)
                    nc.gpsimd.tensor_tensor(
                        out=l13, in0=x3[:, :, 0:N // 2],
                        in1=x3[:, :, N // 2:N], op=Alu.max)
                    l2 = scp.tile([P, G * (N // 4)], f32, tag="l2")
                    l23 = l2[:].rearrange("p (r n) -> p r n", n=N // 4)
                    nc.gpsimd.tensor_tensor(
                        out=l23, in0=l13[:, :, 0:N // 4],
                        in1=l13[:, :, N // 4:N // 2], op=Alu.max)
                    nc.vector.tensor_reduce(out=mg, in_=l23, axis=Ax.X,
                                            op=Alu.max)
                elif max_tree == 1:
                    l1 = scp.tile([P, G * (N // 2)], f32, tag="l1")
                    l13 = l1[:].rearrange("p (r n) -> p r n", n=N // 2)
                    nc.gpsimd.tensor_tensor(
                        out=l13, in0=x3[:, :, 0:N // 2],
                        in1=x3[:, :, N // 2:N], op=Alu.max)
                    nc.vector.tensor_reduce(out=mg, in_=l13, axis=Ax.X,
                                            op=Alu.max)
                elif max_tree == 4:
                    # ttr pairwise max per tile on DVE (dual-read+reduce)
                    l1 = scp.tile([P, G * (N // 2)], f32, tag="l1")
                    for g in range(G):
                        nc.vector.tensor_tensor_reduce(
                            out=l1[:, g * (N // 2):(g + 1) * (N // 2)],
                            in0=x3[:, g, 0:N // 2], in1=x3[:, g, N // 2:N],
                            scale=1.0, scalar=-3.0e38,
                            op0=Alu.max, op1=Alu.max,
                            accum_out=mg[:, g:g + 1])
                elif max_tree == 5:
                    # DVE pairwise tt L1 (2x_2p probe) + L2 reduce
                    l1 = scp.tile([P, G * (N // 2)], f32, tag="l1")
                    l13 = l1[:].rearrange("p (r n) -> p r n", n=N // 2)
                    nc.vector.tensor_tensor(
                        out=l13, in0=x3[:, :, 0:N // 2],
                        in1=x3[:, :, N // 2:N], op=Alu.max)
                    nc.vector.tensor_reduce(out=mg, in_=l13, axis=Ax.X,
                                            op=Alu.max)
                else:
                    nc.vector.tensor_reduce(out=mg, in_=x3, axis=Ax.X,
                                            op=Alu.max)
                rg = Rv[:, cols]
                nc.vector.reciprocal(rg, mg)

                ut = up.tile([P, G * N], dt_u, tag="u")
                if fast:
                    # fused pairs: per-tile exp with scale=r, bias=-1
                    for t in range(2 * k_f):
                        fs = slice(t * N, (t + 1) * N)
                        j = c * G + t
                        nc.scalar.activation(
                            out=ut[:, fs], in_=xt[:, fs], func=Act.Exp,
                            scale=rg[:, t:t + 1], bias=BIAS[:, 0:1])
                if nb:
                    # normalized span: pairs k_f..H
                    t0 = 2 * k_f          # first tile of span
                    yt = yp.tile([P, nb * 2 * N], f32, tag="y")
                    y3 = yt[:].rearrange("p (r n) -> p r n", n=N)
                    xs3 = x3[:, t0:G, :]
                    rb_g = rg[:, t0:t0 + 2 * k_g, None].broadcast_to(
                        [P, 2 * k_g, N])
                    rb_q = rg[:, t0 + 2 * k_g:G, None].broadcast_to(
                        [P, G - t0 - 2 * k_g, N])
                    if k_g:
                        nc.gpsimd.tensor_tensor(
                            out=y3[:, 0:2 * k_g, :],
                            in0=xs3[:, 0:2 * k_g, :], in1=rb_g, op=Alu.mult)
                    if G - t0 - 2 * k_g:
                        nc.vector.tensor_tensor(
                            out=y3[:, 2 * k_g:, :],
                            in0=xs3[:, 2 * k_g:, :], in1=rb_q, op=Alu.mult)
                    if fast:
                        nc.scalar.activation(
                            out=ut[:, t0 * N:], in_=yt[:], func=Act.Exp,
                            scale=1.0, bias=BIAS[:, 0:1])

                if fast:
                    # bn_stats per pair, strided interleaved input
                    for h in range(H):
                        j2 = c * H + h
                        in3 = ut[:, 2 * h * N:(2 * h + 2) * N].rearrange(
                            "p (r n) -> p n r", r=2)
                        nc.vector.add_instruction(
                            mybir.InstBNStats(
                                name=f"I-{nc.next_id()}",
                                ins=[nc.vector.lower_ap(in3)],
                                outs=[nc.vector.lower_ap(
                                    BST[:, j2 * 6:(j2 + 1) * 6])],
                            ))
                else:
                    nc.scalar.activation(
                        out=ut[:], in_=yt[:], func=Act.Exp,
                        scale=float(s), bias=BIAS[:, 0:1])
                    nc.vector.tensor_reduce(
                        out=S2[:, cols],
                        in_=ut[:].rearrange("p (r n) -> p r n", n=N),
                        axis=Ax.X, op=Alu.add)
                    nc.scalar.activation(
                        out=ut[:], in_=yt[:], func=Act.Exp,
                        scale=1.0 + float(s), bias=BIAS[:, 1:2])
                    nc.vector.tensor_reduce(
                        out=S1[:, cols],
                        in_=ut[:].rearrange("p (r n) -> p r n", n=N),
                        axis=Ax.X, op=Alu.add)

            # ---- final: pos = m*S1/S2 ; out = m>0 ? pos : m (m==0 -> 0)
            T2 = T // 2
            FT = statp.tile([P, 3 * T2], f32, tag="FT")
            POS = statp.tile([P, T], f32, tag="POS")
            RO = statp.tile([P, T], f32, tag="RO")
            MK = statp.tile([P, T], mybir.dt.uint8, tag="MK")

            if fast:
                B3 = BST[:].rearrange("p (HH s) -> p HH s", s=6)
                Mv = M[:].rearrange("p (HH two) -> p HH two", two=2)
                Pv = POS[:].rearrange("p (HH two) -> p HH two", two=2)
                t1 = FT[:, 0 * T2:1 * T2]
                As = FT[:, 1 * T2:2 * T2]
                imu = FT[:, 2 * T2:3 * T2]
                for pi in (0, 1):
                    mu = B3[:, :, 1 + 3 * pi]
                    m2 = B3[:, :, 2 + 3 * pi]
                    nc.vector.tensor_tensor(t1, mu, mu, op=Alu.mult)
                    # As = m2/N + mu^2 (= S1/N; S2/N = mu)
                    nc.vector.scalar_tensor_tensor(
                        out=As, in0=m2, scalar=1.0 / float(N), in1=t1,
                        op0=Alu.mult, op1=Alu.add)
                    nc.vector.reciprocal(imu, mu)
                    nc.vector.tensor_tensor(As, As, imu, op=Alu.mult)
                    nc.vector.tensor_tensor(
                        Pv[:, :, pi], As, Mv[:, :, pi], op=Alu.mult)
            else:
                nc.vector.reciprocal(POS[:], S2[:])
                nc.vector.tensor_tensor(POS[:], S1[:], POS[:], op=Alu.mult)
                nc.vector.tensor_tensor(POS[:], POS[:], M[:], op=Alu.mult)

            nc.vector.tensor_scalar(MK[:], M[:], 0.0, None, op0=Alu.is_gt)
            nc.vector.tensor_copy(RO[:], M[:])
            nc.vector.copy_predicated(out=RO[:], mask=MK[:], data=POS[:])

            # store: out row l = c*(G*P) + p*G + r  -> RO[p, c*G+r]
            nc.sync.dma_start(
                out=out_d[:].rearrange("(c p r) -> p c r", p=P, r=G),
                in_=RO[:].rearrange("p (c r) -> p c r", r=G))

    nc.compile()
    return nc


def _run(x: np.ndarray, scale: np.ndarray, trace: bool = False,
         build_kw: dict | None = None, **kw):
    from concourse.bass_utils import run_bass_kernel_spmd

    n_cores = 8
    B, Tm, X, Nn = x.shape          # 32, 256, 64, 256
    assert Nn == N
    rows = B * Tm * X
    rows_per_core = rows // n_cores
    s = float(np.asarray(scale))

    nc = _build(rows_per_core, s, **(build_kw or {}))
    xs = np.ascontiguousarray(np.asarray(x, dtype=np.float32)).reshape(
        n_cores, rows_per_core, N)
    in_maps = [{"x": xs[i]} for i in range(n_cores)]
    res = run_bass_kernel_spmd(nc, in_maps, list(range(n_cores)),
                               trace=trace, **kw)
    out = np.concatenate([r["out"].reshape(-1) for r in res.results], axis=0)
    return out.reshape(B, Tm, X).astype(np.float32), res


def kernel(x: np.ndarray, scale: np.ndarray) -> np.ndarray:
    return _run(x, scale)[0]


# revision 12
# speedup vs baseline: 1.0237x; 1.0237x over previous
"""Maxish pooling kernel for Trainium2 (8 NeuronCores, data-parallel).

Reference math (per row of length N):
    m  = max(x)
    rt = (x - m) / (m + 1e-8)
    pos = m * sum(exp(2*rt)) / sum(exp(rt))     # for scale s == 1
    out = m > 0 ? pos : (m < 0 ? m : 0)

Identity: rt = x*(1/m) - 1 exactly, so u = exp(rt) = Exp(y - 1) with
y = x*r.  Both sums come from one bn_stats pass per row-pair over u
read pair-interleaved (even/odd streams -> per-row mean/M2):
sum u = N*mu, sum u^2 = M2 + N*mu^2.

Work split (per chunk of [128, G=16 rows, 256], all rates measured):
  gpsimd: pairwise-max tree levels 1+2 (tensor_tensor max), plus the
          broadcast normalize y = x*r for `k_g` row-pairs
  DVE:    max tree level 3 (tensor_reduce), reciprocal, normalize for
          `k_q` pairs, bn_stats for all pairs
  ACT:    fused exp (scale=r, bias=-1, per tile) for `k_f` pairs; one
          big contiguous exp over the remaining span
Layout: chunk = 2048 consecutive rows; partition p holds rows
c*2048 + p*16 + r (16KB contiguous per partition line).
"""

import numpy as np

P = 128
N = 256


def _build(n_rows: int, s: float, G: int = 16, x_bufs: int = 3,
           y_bufs: int = 2, u_bufs: int = 2, sc_bufs: int = 2,
           k_f: int = 4, k_g: int = 4,
           dt_u_str: str = "bf16", dt_bst_str: str = "f32",
           max_tree: int = 0):
    from concourse import bacc, mybir
    from concourse.tile import TileContext

    f32 = mybir.dt.float32
    dts = {"f32": mybir.dt.float32, "bf16": mybir.dt.bfloat16}
    dt_u = dts[dt_u_str]
    dt_bst = dts[dt_bst_str]
    Act = mybir.ActivationFunctionType
    Alu = mybir.AluOpType
    Ax = mybir.AxisListType

    assert n_rows % (P * G) == 0
    T = n_rows // P          # rows per partition
    C = T // G               # chunks
    H = G // 2               # row-pairs per chunk
    fast = (s == 1.0)
    if not fast:
        k_f = 0              # generic path: no fused tiles
    k_f = min(k_f, H)
    k_g = min(k_g, H - k_f)  # gpsimd-normalized pairs
    # remaining pairs use DVE normalize
    nb = H - k_f             # pairs needing y + big exp

    nc = bacc.Bacc("TRN2", target_bir_lowering=False, debug=False,
                   num_devices=8)
    x_d = nc.declare_dram_parameter("x", [n_rows, N], f32, isOutput=False)
    out_d = nc.declare_dram_parameter("out", [n_rows], f32, isOutput=True)

    with TileContext(nc) as tc:
        with (
            tc.tile_pool(name="xp", bufs=x_bufs) as xp,
            tc.tile_pool(name="yp", bufs=y_bufs) as yp,
            tc.tile_pool(name="up", bufs=u_bufs) as up,
            tc.tile_pool(name="scp", bufs=sc_bufs) as scp,
            tc.tile_pool(name="stat", bufs=1) as statp,
        ):
            M = statp.tile([P, T], f32, tag="M")
            Rv = statp.tile([P, T], f32, tag="Rv")
            BST = statp.tile([P, (T // 2) * 6], dt_bst, tag="BST")
            BIAS = statp.tile([P, 2], f32, tag="BIAS")
            nc.vector.memset(BIAS[:, 0:1], -float(s))
            nc.vector.memset(BIAS[:, 1:2], -(1.0 + float(s)))
            if not fast:
                S1 = statp.tile([P, T], f32, tag="S1")
                S2 = statp.tile([P, T], f32, tag="S2")

            for c in range(C):
                cols = slice(c * G, (c + 1) * G)
                xt = xp.tile([P, G * N], f32, tag="x")
                src = x_d[c * G * P:(c + 1) * G * P, :].rearrange(
                    "(p r) n -> p r n", p=P)
                nc.sync.dma_start(
                    out=xt[:].rearrange("p (r n) -> p r n", n=N), in_=src)
                x3 = xt[:].rearrange("p (r n) -> p r n", n=N)

                mg = M[:, cols]
                if max_tree == 2:
                    l1 = scp.tile([P, G * (N // 2)], f32, tag="l1")
                    l13 = l1[:].rearrange("p (r n) -> p r n", n=N // 2)
                    nc.gpsimd.tensor_tensor(
                        out=l13, in0=x3[:, :, 0:N // 2],
                        in1=x3[:, :, N // 2:N], op=Alu.max)
                    l2 = scp.tile([P, G * (N // 4)], f32, tag="l2")
                    l23 = l2[:].rearrange("p (r n) -> p r n", n=N // 4)
                    nc.gpsimd.tensor_tensor(
                        out=l23, in0=l13[:, :, 0:N // 4],
                        in1=l13[:, :, N // 4:N // 2], op=Alu.max)
                    nc.vector.tensor_reduce(out=mg, in_=l23, axis=Ax.X,
                                            op=Alu.max)
                elif max_tree == 1:
                    l1 = scp.tile([P, G * (N // 2)], f32, tag="l1")
                    l13 = l1[:].rearrange("p (r n) -> p r n", n=N // 2)
                    nc.gpsimd.tensor_tensor(
                        out=l13, in0=x3[:, :, 0:N // 2],
                        in1=x3[:, :, N // 2:N], op=Alu.max)
                    nc.vector.tensor_reduce(out=mg, in_=l13, axis=Ax.X,
                                            op=Alu.max)
                elif max_tree == 4:
                    # ttr pairwise max per tile on DVE (dual-read+reduce)
                    l1 = scp.tile([P, G * (N // 2)], f32, tag="l1")
                    for g in range(G):
                        nc.vector.tensor_tensor_reduce(
                            out=l1[:, g * (N // 2):(g + 1) * (N // 2)],
                            in0=x3[:, g, 0:N // 2], in1=x3[:, g, N // 2:N],
                            scale=1.0, scalar=-3.0e38,
                            op0=Alu.max, op1=Alu.max,
                            accum_out=mg[:, g:g + 1])
                elif max_tree == 5:
                    # DVE pairwise tt L1 (2x_2p probe) + L2 reduce
                    l1 = scp.tile([P, G * (N // 2)], f32, tag="l1")
                    l13 = l1[:].rearrange("p (r n) -> p r n", n=N // 2)
                    nc.vector.tensor_tensor(
                        out=l13, in0=x3[:, :, 0:N // 2],
                        in1=x3[:, :, N // 2:N], op=Alu.max)
                    nc.vector.tensor_reduce(out=mg, in_=l13, axis=Ax.X,
                                            op=Alu.max)
                else:
                    nc.vector.tensor_reduce(out=mg, in_=x3, axis=Ax.X,
                                            op=Alu.max)
                rg = Rv[:, cols]
                nc.vector.reciprocal(rg, mg)

                ut = up.tile([P, G * N], dt_u, tag="u")
                if fast:
                    # fused pairs: per-tile exp with scale=r, bias=-1
                    for t in range(2 * k_f):
                        fs = slice(t * N, (t + 1) * N)
                        j = c * G + t
                        nc.scalar.activation(
                            out=ut[:, fs], in_=xt[:, fs], func=Act.Exp,
                            scale=rg[:, t:t + 1], bias=BIAS[:, 0:1])
                if nb:
                    # normalized span: pairs k_f..H
                    t0 = 2 * k_f          # first tile of span
                    yt = yp.tile([P, nb * 2 * N], f32, tag="y")
                    y3 = yt[:].rearrange("p (r n) -> p r n", n=N)
                    xs3 = x3[:, t0:G, :]
                    rb_g = rg[:, t0:t0 + 2 * k_g, None].broadcast_to(
                        [P, 2 * k_g, N])
                    rb_q = rg[:, t0 + 2 * k_g:G, None].broadcast_to(
                        [P, G - t0 - 2 * k_g, N])
                    if k_g:
                        nc.gpsimd.tensor_tensor(
                            out=y3[:, 0:2 * k_g, :],
                            in0=xs3[:, 0:2 * k_g, :], in1=rb_g, op=Alu.mult)
                    if G - t0 - 2 * k_g:
                        nc.vector.tensor_tensor(
                            out=y3[:, 2 * k_g:, :],
                            in0=xs3[:, 2 * k_g:, :], in1=rb_q, op=Alu.mult)
                    if fast:
                        nc.scalar.activation(
                            out=ut[:, t0 * N:], in_=yt[:], func=Act.Exp,
                            scale=1.0, bias=BIAS[:, 0:1])

                if fast:
                    # bn_stats per pair, strided interleaved input
                    for h in range(H):
                        j2 = c * H + h
                        in3 = ut[:, 2 * h * N:(2 * h + 2) * N].rearrange(
                            "p (r n) -> p n r", r=2)
                        nc.vector.add_instruction(
                            mybir.InstBNStats(
                                name=f"I-{nc.next_id()}",
                                ins=[nc.vector.lower_ap(in3)],
                                outs=[nc.vector.lower_ap(
                                    BST[:, j2 * 6:(j2 + 1) * 6])],
                            ))
                else:
                    nc.scalar.activation(
                        out=ut[:], in_=yt[:], func=Act.Exp,
                        scale=float(s), bias=BIAS[:, 0:1])
                    nc.vector.tensor_reduce(
                        out=S2[:, cols],
                        in_=ut[:].rearrange("p (r n) -> p r n", n=N),
                        axis=Ax.X, op=Alu.add)
                    nc.scalar.activation(
                        out=ut[:], in_=yt[:], func=Act.Exp,
                        scale=1.0 + float(s), bias=BIAS[:, 1:2])
                    nc.vector.tensor_reduce(
                        out=S1[:, cols],
                        in_=ut[:].rearrange("p (r n) -> p r n", n=N),
                        axis=Ax.X, op=Alu.add)

            # ---- final: pos = m*S1/S2 ; out = m>0 ? pos : m (m==0 -> 0)
            T2 = T // 2
            FT = statp.tile([P, 3 * T2], f32, tag="FT")
            POS = statp.tile([P, T], f32, tag="POS")
            RO = statp.tile([P, T], f32, tag="RO")
            MK = statp.tile([P, T], mybir.dt.uint8, tag="MK")

            if fast:
                B3 = BST[:].rearrange("p (HH s) -> p HH s", s=6)
                Mv = M[:].rearrange("p (HH two) -> p HH two", two=2)
                Pv = POS[:].rearrange("p (HH two) -> p HH two", two=2)
                t1 = FT[:, 0 * T2:1 * T2]
                As = FT[:, 1 * T2:2 * T2]
                imu = FT[:, 2 * T2:3 * T2]
                for pi in (0, 1):
                    mu = B3[:, :, 1 + 3 * pi]
                    m2 = B3[:, :, 2 + 3 * pi]
                    nc.vector.tensor_tensor(t1, mu, mu, op=Alu.mult)
                    # As = m2/N + mu^2 (= S1/N; S2/N = mu)
                    nc.vector.scalar_tensor_tensor(
                        out=As, in0=m2, scalar=1.0 / float(N), in1=t1,
                        op0=Alu.mult, op1=Alu.add)
                    nc.vector.reciprocal(imu, mu)
                    nc.vector.tensor_tensor(As, As, imu, op=Alu.mult)
                    nc.vector.tensor_tensor(
                        Pv[:, :, pi], As, Mv[:, :, pi], op=Alu.mult)
            else:
                nc.vector.reciprocal(POS[:], S2[:])
                nc.vector.tensor_tensor(POS[:], S1[:], POS[:], op=Alu.mult)
                nc.vector.tensor_tensor(POS[:], POS[:], M[:], op=Alu.mult)

            nc.vector.tensor_scalar(MK[:], M[:], 0.0, None, op0=Alu.is_gt)
            nc.vector.tensor_copy(RO[:], M[:])
            nc.vector.copy_predicated(out=RO[:], mask=MK[:], data=POS[:])

            # store: out row l = c*(G*P) + p*G + r  -> RO[p, c*G+r]
            nc.sync.dma_start(
                out=out_d[:].rearrange("(c p r) -> p c r", p=P, r=G),
                in_=RO[:].rearrange("p (c r) -> p c r", r=G))

    nc.compile()
    return nc


def _run(x: np.ndarray, scale: np.ndarray, trace: bool = False,
         build_kw: dict | None = None, **kw):
    from concourse.bass_utils import run_bass_kernel_spmd

    n_cores = 8
    B, Tm, X, Nn = x.shape          # 32, 256, 64, 256
    assert Nn == N
    rows = B * Tm * X
    rows_per_core = rows // n_cores
    s = float(np.asarray(scale))

    nc = _build(rows_per_core, s, **(build_kw or {}))
    xs = np.ascontiguousarray(np.asarray(x, dtype=np.float32)).reshape(
        n_cores, rows_per_core, N)
    in_maps = [{"x": xs[i]} for i in range(n_cores)]
    res = run_bass_kernel_spmd(nc, in_maps, list(range(n_cores)),
                               trace=trace, **kw)
    out = np.concatenate([r["out"].reshape(-1) for r in res.results], axis=0)
    return out.reshape(B, Tm, X).astype(np.float32), res


def kernel(x: np.ndarray, scale: np.ndarray) -> np.ndarray:
    return _run(x, scale)[0]
